# revision 21
# baseline (speedup 1.0000x reference)
"""Grouped fp8 block-quantized GEMM (DeepSeekV3 GroupColumnParallelLinear) on 8 trn2 cores.

Math per group g (G=8, T=1024, K=7168, N=2048, BLOCK=128):
  a_scale[t,kb] = max|x[t, kb*128:(kb+1)*128]| / 448
  x_deq = fp8_e4m3fn_rne(x / a_scale) * a_scale
  w_deq = weight * scale (per 128x128 block)
  y = x_deq @ w_deq.T + bias     (fp32 accumulation)

Sharding: one group per NeuronCore (expert parallel, zero communication).

Host prep (exact reference math in fp32, folded into operand layout):
  - w_deq = weight * scale precomputed fp32, rounded once to bf16 (the
    matmul operand precision), laid out K-major [16 nt][128 k][56 kb][128 n].
  - x_deq = fp8_e4m3fn(x / a_scale) * a_scale precomputed fp32 (bit-exact
    reference act-quant), rounded once to bf16, pre-transposed to
    k-partition quad layout [2 t-half][14 quad][128 k][4 kb][512 t].
  - bias laid out [128, 16] so each n-tile's bias is a per-partition vector.

Device kernel per core: pure w-stationary bf16 GEMM — per (t-half, nt):
  56 matmuls of 512 cols accumulate psum[n128, t512] over kb; bias add on
  PSUM->SBUF eviction (scalar activation); y[n, t] DMA out (host transposes
  back).  PE stream is nothing but the 1792 roofline matmuls; x/y DMA on
  the sync ring, w DMA on the scalar ring, 4-deep weight prefetch.
"""

import os
import sys

import numpy as np

for _p in ("/opt/trn_rl_repo",):
    if _p not in sys.path and os.path.isdir(_p):
        sys.path.insert(0, _p)

import ml_dtypes  # noqa: E402

G, T, K, N = 8, 1024, 7168, 2048
P = 128
KB = K // P  # 56
NT = N // P  # 16
TH = T // 2  # 512
NQ = KB // 4  # 14 quad tiles per t-half
FP8_MAX = 448.0

_NC_CACHE = {}


def _build_nc():
    import concourse.bacc as bacc
    import concourse.mybir as mybir
    import concourse.tile as tile

    dt = mybir.dt
    nc = bacc.Bacc("TRN2", target_bir_lowering=False, debug=False)

    xT_d = nc.dram_tensor("xt", [2, NQ, P, 4, TH], dt.bfloat16, kind="ExternalInput")
    w_d = nc.dram_tensor("w", [NT, P, KB, P], dt.bfloat16, kind="ExternalInput")
    b_d = nc.dram_tensor("b", [P, NT], dt.float32, kind="ExternalInput")
    y_d = nc.dram_tensor("y", [N, T], dt.float32, kind="ExternalOutput")

    AF = mybir.ActivationFunctionType

    with tile.TileContext(nc) as tc:
        with (
            tc.tile_pool(name="const", bufs=1) as const,
            tc.tile_pool(name="xT0", bufs=1) as xT0_p,
            tc.tile_pool(name="xT1", bufs=1) as xT1_p,
            tc.tile_pool(name="wsb", bufs=4) as wsb_p,
            tc.tile_pool(name="ysb", bufs=3) as ysb_p,
            tc.tile_pool(name="mpsum", bufs=7, space="PSUM") as mps_p,
            tc.tile_pool(name="wpsum", bufs=1, space="PSUM") as wps_p,
        ):
            bias_sb = const.tile([P, NT], dt.float32)
            nc.sync.dma_start(bias_sb[:], b_d[:, :])
            warm_src = const.tile([P, TH], dt.bfloat16)
            nc.vector.memset(warm_src[:], 0.0)
            warm_ps = wps_p.tile([P, TH], dt.float32, name="warmps")

            xT = [
                [
                    half_p.tile([P, 4, TH], dt.bfloat16, name=f"xT{h}_{q}")
                    for q in range(NQ)
                ]
                for h, half_p in enumerate((xT0_p, xT1_p))
            ]

            def load_quad(h, q):
                nc.sync.dma_start(xT[h][q][:], xT_d[h, q, :, :, :])

            def mm_chain(h, nt, w=None):
                tcol = h * TH
                if w is None:
                    w = wsb_p.tile([P, KB, P], dt.bfloat16, name="wsb")
                    nc.scalar.dma_start(
                        w[:, 0 : KB // 2, :], w_d[nt, :, 0 : KB // 2, :]
                    )
                    nc.scalar.dma_start(
                        w[:, KB // 2 : KB, :], w_d[nt, :, KB // 2 : KB, :]
                    )
                ps = mps_p.tile([P, TH], dt.float32, name="mpsum")
                for kb in range(KB):
                    nc.tensor.matmul(
                        ps[:],
                        w[:, kb, :],
                        xT[h][kb // 4][:, kb % 4, :],
                        start=(kb == 0),
                        stop=(kb == KB - 1),
                    )
                y = ysb_p.tile([P, TH], dt.float32, name="ysb")
                nc.scalar.activation(
                    y[:], ps[:], AF.Identity, bias=bias_sb[:, nt : nt + 1]
                )
                nc.sync.dma_start(
                    y_d[nt * P : (nt + 1) * P, tcol : tcol + TH], y[:]
                )

            # w0/w1 on the sync ring AHEAD of the quads: the crawl is
            # weight-starved (each quad arrival unlocks work only on chains
            # whose w has landed), so front-loading two weight tiles plus the
            # scalar ring's w2+ gets ~7 chains runnable as quads stream in
            wpre = []
            for nt in range(2):
                w = wsb_p.tile([P, KB, P], dt.bfloat16, name="wsb")
                nc.sync.dma_start(w[:, 0 : KB // 2, :], w_d[nt, :, 0 : KB // 2, :])
                nc.sync.dma_start(w[:, KB // 2 : KB, :], w_d[nt, :, KB // 2 : KB, :])
                wpre.append(w)
            for q in range(NQ):
                load_quad(0, q)
            for nt in range(NT):
                mm_chain(0, nt, w=wpre[nt] if nt < 2 else None)
                if nt < NQ:
                    load_quad(1, nt)
            for nt in range(NT):
                mm_chain(1, nt)

            # dep-free PE warmups, issued last = lowest scheduler priority:
            # they fill the DMA head so the HAM clock-gate is at 8/8 when
            # the real chains start
            for _ in range(24):
                nc.tensor.matmul(
                    warm_ps[:], warm_src[:, 0:P], warm_src[:], start=True, stop=True
                )

    nc.compile()
    return nc


def _get_nc():
    if "nc" not in _NC_CACHE:
        _NC_CACHE["nc"] = _build_nc()
    return _NC_CACHE["nc"]


def _prep_inputs(xs, weight, scale, bias):
    bf16 = ml_dtypes.bfloat16
    f8 = ml_dtypes.float8_e4m3fn
    in_maps = []
    for g in range(G):
        # --- exact reference act-quant in fp32, then one bf16 rounding ---
        xb = np.ascontiguousarray(xs[g], dtype=np.float32).reshape(T, KB, P)
        a_scale = np.max(np.abs(xb), axis=-1) / FP8_MAX  # [T, KB]
        x_q = (xb / a_scale[:, :, None]).astype(f8).astype(np.float32)
        x_deq = (x_q * a_scale[:, :, None]).astype(bf16)  # [T, KB, P]
        # [t, kb, p] -> [2, 14, 128 k-part, 4 kb, 512 t]
        x_host = np.ascontiguousarray(
            x_deq.reshape(2, TH, NQ, 4, P).transpose(0, 2, 4, 3, 1)
        )
        # --- fold per-block scale into the fp8 code values ---
        w_deq = (
            weight[g].reshape(NT, P, KB, P)
            * scale[g].astype(np.float32)[:, None, :, None]
        ).astype(bf16)
        # [nt, n1, kb, p] -> [nt, p, kb, n1]  (k-partition-major for DMA)
        w_host = np.ascontiguousarray(w_deq.transpose(0, 3, 2, 1))
        b_host = np.ascontiguousarray(bias[g].reshape(NT, P).T.astype(np.float32))
        in_maps.append({"xt": x_host, "w": w_host, "b": b_host})
    return in_maps


def _install_ntff_shim():
    # this trimmed image lacks ``antenv.axon_hooks``; recreate it so
    # run_bass_kernel_spmd(trace=True) can reach the axon NTFF profiler
    import types

    if "antenv.axon_hooks" in sys.modules:
        return
    try:
        if "/root/.axon_site" not in sys.path:
            sys.path.insert(0, "/root/.axon_site")
        from trn_agent_boot.trn_boot import _ntff_profile_via_ctypes

        hook = _ntff_profile_via_ctypes("/opt/axon/libaxon_pjrt.so")
    except Exception:
        hook = None
    mod = types.ModuleType("antenv.axon_hooks")
    mod._hook = hook
    mod.get_axon_ntff_profile_hook = lambda: mod._hook
    mod.set_axon_ntff_profile_hook = lambda h: setattr(mod, "_hook", h)
    sys.modules["antenv.axon_hooks"] = mod
    try:
        import antenv

        antenv.axon_hooks = mod
    except Exception:
        pass


def kernel(xs, weight, scale, bias, _trace=False, _tmpdir=None):
    from concourse.bass_utils import run_bass_kernel_spmd

    if _trace:
        _install_ntff_shim()

    nc = _get_nc()
    in_maps = _prep_inputs(xs, weight, scale, bias)
    res = run_bass_kernel_spmd(
        nc, in_maps, list(range(G)), trace=_trace, tmpdir=_tmpdir
    )
    out = np.stack([r["y"].T for r in res.results]).astype(np.float32)
    if _trace:
        kernel.last_results = res
    return out
